# revision 27
# baseline (speedup 1.0000x reference)
"""SSIM(3x3 avg-pool) + L1 loss kernel for Trainium2, 8 NeuronCores.

loss = 0.85 * mean(clip((1 - ssim_map)/2, 0, 1)) + 0.15 * mean(|pred - target|)

Full inputs pred/target: (16, 1, 1024, 1024) f32. Data-parallel: 2 images per
core; each core returns per-partition partial sums [128, 2] (col 0 = sum of
the clipped ssim loss map, col 1 = sum |pred-target|); the host combines and
applies the means / alpha-beta weights.

Math (per image pair, variance identities halve the pooled field count):
  u = p + t, v = p - t
  box(x) = 3x3 zero-padded box sum / 9 (separable)
  X = box(p), Y = box(t), G = box(u^2), Hh = box(v^2)
  n1*n2 = (2XY + C1) * ((G-Hh)/2 - 2XY + C2)
  d1*d2 = (X^2+Y^2 + C1) * ((G+Hh)/2 - (X^2+Y^2) + C2)
  ssim_map = n1*n2/(d1*d2);  contrib = clip(0.5 - 0.5*ssim_map, 0, 1)
  l1 from |v|.

Layout: the host packs each image pair row-wise as [0 p0 0|0 p1 0|0 t0 0|0 t1 0]
(zero-padded sections of width W+2), so every pre-pool stage (horizontal 3-tap
via shifted adds, u/v, squares, |v|) is ONE wide instruction across both
images and both tensors. The vertical 3-tap runs on the TensorEngine as a
banded [128 -> <=126] float32r matmul per 512-col chunk into a single 8-bank
PSUM tile; row halos come from overlapped 128-row stripes, image edges from
per-block banded matrices. Post-pool math uses custom fused DVE ops
(x^2+y^2, the (a+c0)(b*c1-a+c2) rational terms, and a fused
clip-and-accumulate), one reciprocal_approx_fast for the division.
"""

import sys

import numpy as np

sys.path.insert(0, "/opt/trn_rl_repo")

ALPHA = 0.85
BETA = 0.15
C1 = 0.01 ** 2
C2 = 0.03 ** 2

N_CORES = 8
IMG_H = 1024
IMG_W = 1024
N_IMG_PER_CORE = 2
BLK = 126          # output rows per vertical-matmul block
MAXW_PSUM = 512    # fp32 columns per PSUM bank

MM_F32R = True     # float32r matmuls: 4x PE throughput, ~1e-6 rel error

# --- custom fused DVE ops (registered into concourse.dve_ops at build) ---- #
_OP_SQSUM = None       # out = in0^2 + in1^2
_OP_SSIM_RAT = None    # out = (in0 + s0) * (in1*s1 - in0 + imm2)
_OP_SSIM_FINAL = None  # out = (s0 - clamp(in0*in1, s1, s0))*imm2; accum += out
_CUSTOM_OPS_OK = False


def _register_custom_ops():
    global _OP_SQSUM, _OP_SSIM_RAT, _OP_SSIM_FINAL, _CUSTOM_OPS_OK
    if _CUSTOM_OPS_OK:
        return
    from operator import add

    import concourse.dve_ops as dv
    from concourse.dve_spec import (
        C0, C1 as SC1, C2 as SC2, Spec, Src0, Src1, Zero, lower, maxx, minn, sq,
    )
    from concourse.dve_uop import DveOpSpec

    def _sqsum_ref(in0, in1, c0, c1, c2):
        return in0.astype(np.float32) ** 2 + in1.astype(np.float32) ** 2

    def _rat_ref(in0, in1, c0, c1, c2):
        a = in0.astype(np.float32)
        return (a + c0) * (in1.astype(np.float32) * c1 - a + c2)

    def _final_ref(in0, in1, c0, c1, c2):
        z = in0.astype(np.float32) * in1.astype(np.float32)
        b = ((c0 - np.clip(z, c1, c0)) * c2).astype(np.float32)
        return b, b.reshape(b.shape[0], -1).sum(axis=-1, keepdims=True)

    defs = [
        ("SSIM_SQSUM_ANT", Spec(body=sq(Src0) + sq(Src1), reference=_sqsum_ref)),
        ("SSIM_RAT_ANT", Spec(
            body=(Src0 + C0) * (Src1 * SC1 - Src0 + SC2), reference=_rat_ref)),
        ("SSIM_FINAL_ANT", Spec(
            body=(C0 - maxx(minn(Src0 * Src1, C0), SC1)) * SC2,
            accum=add, accum_init=Zero, reference=_final_ref)),
    ]
    made = {}
    for name, spec in defs:
        if name not in dv._SUB_OPCODE_FOR_NAME:
            stub = dv.DveOp(name, spec, subdim=False, uops_sha={})
            dv.OPS.append(stub)
            dv._SUB_OPCODE_FOR_NAME[name] = (
                dv._CUSTOM_DVE_ROW_BASE + len(dv.OPS) - 1
            )
            dv.CUSTOM_DVE_SPECS[name] = spec
        opcode = dv._SUB_OPCODE_FOR_NAME[name]
        shas = {}
        for ver in ("v3", "v4"):
            res = DveOpSpec(
                name=name, opcode=opcode, uops=lower(spec, ver=ver),
                rd1_en=dv.has_src1(spec),
            )
            shas[ver] = res.sha(ver)
        op = dv.DveOp(name, spec, subdim=False, uops_sha=shas)
        idx = next(i for i, o in enumerate(dv.OPS) if o.name == name)
        dv.OPS[idx] = op
        dv.CUSTOM_DVE_SPECS[name] = spec
        made[name] = op
    _OP_SQSUM = made["SSIM_SQSUM_ANT"]
    _OP_SSIM_RAT = made["SSIM_RAT_ANT"]
    _OP_SSIM_FINAL = made["SSIM_FINAL_ANT"]
    _CUSTOM_OPS_OK = True


def _blocks(H):
    """Vertical block decomposition: list of (r0, n_out, rs, nr)."""
    out = []
    b = 0
    while b * BLK < H:
        r0 = b * BLK
        n_out = min(BLK, H - r0)
        rs = max(r0 - 1, 0)
        re = min(r0 + n_out, H - 1)
        out.append((r0, n_out, rs, re - rs + 1))
        b += 1
    return out


def make_bmats(H):
    """Banded vertical-sum matrices (entries 1/9), padded into [nblk,128,BLK]."""
    blocks = _blocks(H)
    bm = np.zeros((len(blocks), 128, BLK), dtype=np.float32)
    ninth = np.float32(1.0) / np.float32(9.0)
    for i, (r0, n_out, rs, nr) in enumerate(blocks):
        for k in range(nr):
            for j in range(n_out):
                if abs((rs + k) - (r0 + j)) <= 1:
                    bm[i, k, j] = ninth
    return bm


def build_program(n_img, H, W, io_internal=False):
    """Build the per-core program for n_img (even) HxW images.

    DRAM input "ptin": [ (n_img/2)*H, 4*(W+2) ] with row layout
    [0 p0 0 | 0 p1 0 | 0 t0 0 | 0 t1 0] per image pair.
    io_internal makes ptin Internal DRAM (timing-only builds).
    """
    import concourse.bacc as bacc
    import concourse.tile as tile
    from concourse import mybir

    assert n_img % 2 == 0
    f32 = mybir.dt.float32
    Alu = mybir.AluOpType
    Act = mybir.ActivationFunctionType

    blocks = _blocks(H)
    nblk = len(blocks)
    S = W + 2                       # one padded section
    S4 = 4 * S                      # packed row width
    npairs = n_img // 2
    n_chunks = (W + MAXW_PSUM - 1) // MAXW_PSUM
    W2 = 2 * W                      # field-pair width (img0|img1)

    _register_custom_ops()
    nc = bacc.Bacc("TRN2", target_bir_lowering=False, debug=False)

    io_kind = "Internal" if io_internal else "ExternalInput"
    ptin_d = nc.dram_tensor("ptin", [npairs * H, S4], f32, kind=io_kind).ap()
    bm_d = nc.dram_tensor("bmats", [nblk, 128, BLK], f32, kind="ExternalInput").ap()
    acc_d = nc.dram_tensor("acc_out", [128, 2], f32, kind="ExternalOutput").ap()

    with tile.TileContext(nc) as tc:
        with (
            tc.tile_pool(name="consts", bufs=1) as cpool,
            tc.tile_pool(name="io", bufs=1) as iopool,
            tc.tile_pool(name="hsum", bufs=1) as hpool,
            tc.tile_pool(name="post", bufs=1) as ppool,
            tc.tile_pool(name="psum", bufs=1, space="PSUM") as psumpool,
        ):
            acc = cpool.tile([128, 2], f32, tag="acc")
            nc.vector.memset(acc[:, :], 0.0)
            if io_internal:
                fill = cpool.tile([128, S4], f32, tag="fill")
                nc.vector.memset(fill[:, :], 0.625)
                rows_total = npairs * H
                for r in range(0, rows_total, 128):
                    nrr = min(128, rows_total - r)
                    nc.sync.dma_start(out=ptin_d[r:r + nrr, :], in_=fill[0:nrr, :])

            mm_dt = mybir.dt.float32r if MM_F32R else f32
            bmats = []
            for i, (r0, n_out, rs, nr) in enumerate(blocks):
                braw = cpool.tile([128, BLK], f32, tag=f"bmraw{i}", name="braw")
                nc.sync.dma_start(out=braw[0:nr, 0:n_out], in_=bm_d[i, 0:nr, 0:n_out])
                if MM_F32R:
                    bt = cpool.tile([128, BLK], mm_dt, tag=f"bmat{i}", name="bt")
                    nc.vector.tensor_copy(bt[0:nr, 0:n_out], braw[0:nr, 0:n_out])
                else:
                    bt = braw
                bmats.append(bt)

            for pair in range(npairs):
                base = pair * H
                for bi, (r0, n_out, rs, nr) in enumerate(blocks):
                    # rows [0:k_l1] of consecutive stripes tile H exactly once
                    if bi + 1 < len(blocks):
                        k_l1 = blocks[bi + 1][2] - rs
                    else:
                        k_l1 = nr

                    pt = iopool.tile([128, S4], f32, tag="pt")
                    nc.sync.dma_start(
                        out=pt[0:nr, :], in_=ptin_d[base + rs: base + rs + nr, :])

                    rows = slice(0, nr)
                    # horizontal 3-tap for p0,p1,t0,t1 in two ops
                    # (junk at section tails is never read)
                    g = hpool.tile([128, S4 - 1], f32, tag="g")
                    nc.vector.tensor_add(
                        g[rows, :], pt[rows, 0:S4 - 1], pt[rows, 1:S4])
                    h3pt = hpool.tile([128, S4 - 2], mm_dt, tag="h3pt")
                    nc.vector.tensor_add(
                        h3pt[rows, :], g[rows, 0:S4 - 2], pt[rows, 2:S4])

                    # in place: t-half <- v = p - t ; p-half <- u = 2p - v
                    nc.vector.tensor_sub(
                        pt[rows, 2 * S:S4], pt[rows, 0:2 * S], pt[rows, 2 * S:S4])
                    nc.vector.scalar_tensor_tensor(
                        pt[rows, 0:2 * S], pt[rows, 0:2 * S], 2.0,
                        pt[rows, 2 * S:S4], op0=Alu.mult, op1=Alu.subtract)
                    # L1 partial: |v| in place over the disjoint-cover rows
                    l1part = ppool.tile([128, 1], f32, tag="l1part")
                    nc.scalar.activation(
                        pt[0:k_l1, 2 * S:S4], pt[0:k_l1, 2 * S:S4], Act.Abs,
                        accum_out=l1part[0:k_l1, :])
                    # squares in place: [u0 u1 v0 v1] -> [u0^2 u1^2 v0^2 v1^2]
                    nc.scalar.activation(pt[rows, :], pt[rows, :], Act.Square)

                    g2 = hpool.tile([128, S4 - 1], f32, tag="g", name="g2")
                    nc.vector.tensor_add(
                        g2[rows, :], pt[rows, 0:S4 - 1], pt[rows, 1:S4])
                    h3uv = hpool.tile([128, S4 - 2], mm_dt, tag="h3uv")
                    nc.vector.tensor_add(
                        h3uv[rows, :], g2[rows, 0:S4 - 2], pt[rows, 2:S4])

                    bmat = bmats[bi]
                    ro = slice(0, n_out)
                    pw = slice(0, W2)

                    def mm_group(h3, ps):
                        # fields [f0_img0|f0_img1|f1_img0|f1_img1] -> PSUM
                        for s in range(4):
                            for ci in range(n_chunks):
                                c0 = ci * MAXW_PSUM
                                cw = min(MAXW_PSUM, W - c0)
                                nc.tensor.matmul(
                                    ps[0:n_out, s * W + c0: s * W + c0 + cw],
                                    lhsT=bmat[0:nr, 0:n_out],
                                    rhs=h3[0:nr, s * S + c0: s * S + c0 + cw],
                                    start=True, stop=True)

                    # group 1: X|Y
                    ps = psumpool.tile([128, 4 * W], f32, tag="ps", name="ps")
                    mm_group(h3pt, ps)
                    Ysb = ppool.tile([128, W2], f32, tag="Ysb", name="Ysb")
                    nc.scalar.copy(Ysb[ro, :], ps[ro, W2:4 * W])
                    A2 = ppool.tile([128, W2], f32, tag="A2", name="A2")
                    nc.vector.scalar_tensor_tensor(
                        A2[ro, pw], ps[ro, 0:W2], 2.0, Ysb[ro, pw],
                        op0=Alu.mult, op1=Alu.mult)
                    V = ppool.tile([128, W2], f32, tag="V", name="V")
                    nc.vector._custom_dve(
                        _OP_SQSUM, out=V[ro, pw], in0=ps[ro, 0:W2], in1=Ysb[ro, pw])

                    # group 2: G|Hh (reuses the PSUM banks)
                    ps2 = psumpool.tile([128, 4 * W], f32, tag="ps", name="ps2")
                    mm_group(h3uv, ps2)
                    Hsb = ppool.tile([128, W2], f32, tag="Hsb", name="Hsb")
                    nc.scalar.copy(Hsb[ro, :], ps2[ro, W2:4 * W])
                    Dd = ppool.tile([128, W2], f32, tag="Dd", name="Dd")
                    nc.vector.tensor_sub(Dd[ro, pw], ps2[ro, 0:W2], Hsb[ro, pw])
                    M = ppool.tile([128, W2], f32, tag="M", name="M")
                    nc.vector.tensor_add(M[ro, pw], ps2[ro, 0:W2], Hsb[ro, pw])

                    # in-place: n1n2 -> A2's tile, d1d2 -> V, rcp -> M, fin -> Dd
                    n1n2 = A2
                    nc.vector._custom_dve(
                        _OP_SSIM_RAT, out=n1n2[ro, pw], in0=A2[ro, pw],
                        in1=Dd[ro, pw], s0=float(C1), s1=0.5, imm2=float(C2))
                    d1d2 = V
                    nc.vector._custom_dve(
                        _OP_SSIM_RAT, out=d1d2[ro, pw], in0=V[ro, pw],
                        in1=M[ro, pw], s0=float(C1), s1=0.5, imm2=float(C2))
                    rcp = M
                    nc.vector.reciprocal_approx_fast(rcp[ro, pw], d1d2[ro, pw])
                    fin = Dd
                    spart = ppool.tile([128, 1], f32, tag="spart")
                    nc.vector._custom_dve(
                        _OP_SSIM_FINAL, out=fin[ro, pw], in0=n1n2[ro, pw],
                        in1=rcp[ro, pw], s0=1.0, s1=-1.0, imm2=0.5,
                        accum_out=spart[ro, :])
                    nc.vector.tensor_add(
                        acc[0:n_out, 0:1], acc[0:n_out, 0:1], spart[ro, :])
                    nc.vector.tensor_add(
                        acc[0:k_l1, 1:2], acc[0:k_l1, 1:2], l1part[0:k_l1, :])

            nc.sync.dma_start(out=acc_d[:, :], in_=acc[:, :])

    nc.compile()
    return nc


_CACHE = {}


def _get_program(n_img, H, W):
    key = (n_img, H, W)
    if key not in _CACHE:
        _CACHE[key] = build_program(n_img, H, W)
    return _CACHE[key]


def _pack_inputs(pred, target):
    """pred/target [n_img, H, W] -> packed [npairs*H, 4*(W+2)]."""
    n_img, H, W = pred.shape
    assert n_img % 2 == 0
    npairs = n_img // 2
    S = W + 2
    out = np.zeros((npairs * H, 4 * S), dtype=np.float32)
    out[:, 1:W + 1] = pred[0::2].reshape(npairs * H, W)
    out[:, S + 1:S + W + 1] = pred[1::2].reshape(npairs * H, W)
    out[:, 2 * S + 1:2 * S + W + 1] = target[0::2].reshape(npairs * H, W)
    out[:, 3 * S + 1:3 * S + W + 1] = target[1::2].reshape(npairs * H, W)
    return out


LAST_RESULTS = None


def kernel(pred, target):
    from concourse.bass_utils import run_bass_kernel_spmd

    global LAST_RESULTS

    pred = np.asarray(pred, dtype=np.float32).reshape(16, IMG_H, IMG_W)
    target = np.asarray(target, dtype=np.float32).reshape(16, IMG_H, IMG_W)

    nc = _get_program(N_IMG_PER_CORE, IMG_H, IMG_W)
    bm = make_bmats(IMG_H)

    in_maps = []
    for c in range(N_CORES):
        sl = slice(c * N_IMG_PER_CORE, (c + 1) * N_IMG_PER_CORE)
        in_maps.append({
            "ptin": _pack_inputs(pred[sl], target[sl]),
            "bmats": bm,
        })

    res = run_bass_kernel_spmd(nc, in_maps, list(range(N_CORES)))
    LAST_RESULTS = res
    ssim_sum = 0.0
    l1_sum = 0.0
    for r in res.results:
        acc = r["acc_out"]
        ssim_sum += float(acc[:, 0].sum(dtype=np.float64))
        l1_sum += float(acc[:, 1].sum(dtype=np.float64))
    n = 16.0 * IMG_H * IMG_W
    loss = ALPHA * (ssim_sum / n) + BETA * (l1_sum / n)
    return np.float32(loss)


# revision 31
# speedup vs baseline: 96.8019x; 96.8019x over previous
"""SSIM(3x3 avg-pool) + L1 loss kernel for Trainium2, 8 NeuronCores.

loss = 0.85 * mean(clip((1 - ssim_map)/2, 0, 1)) + 0.15 * mean(|pred - target|)

Full inputs pred/target: (16, 1, 1024, 1024) f32. Data-parallel: 2 images per
core; each core returns per-partition partial sums [128, 2] (col 0 = sum of
the clipped ssim loss map, col 1 = sum |pred-target|); the host combines and
applies the means / alpha-beta weights.

Math (per image pair, variance identities halve the pooled field count):
  u = p + t, v = p - t
  box(x) = 3x3 zero-padded box sum / 9 (separable)
  X = box(p), Y = box(t), G = box(u^2), Hh = box(v^2)
  n1*n2 = (2XY + C1) * ((G-Hh)/2 - 2XY + C2)
  d1*d2 = (X^2+Y^2 + C1) * ((G+Hh)/2 - (X^2+Y^2) + C2)
  ssim_map = n1*n2/(d1*d2);  contrib = clip(0.5 - 0.5*ssim_map, 0, 1)
  l1 from |v|.

Layout: the host packs each image pair row-wise as [0 p0 0|0 p1 0|0 t0 0|0 t1 0]
(zero-padded sections of width W+2), so every pre-pool stage (horizontal 3-tap
via shifted adds, u/v, squares, |v|) is ONE wide instruction across both
images and both tensors. The vertical 3-tap runs on the TensorEngine as a
banded [128 -> <=126] float32r matmul per 512-col chunk into a single 8-bank
PSUM tile; row halos come from overlapped 128-row stripes, image edges from
per-block banded matrices. Post-pool math uses custom fused DVE ops
(x^2+y^2, the (a+c0)(b*c1-a+c2) rational terms, and a fused
clip-and-accumulate), one reciprocal_approx_fast for the division.
"""

import sys

import numpy as np

sys.path.insert(0, "/opt/trn_rl_repo")

ALPHA = 0.85
BETA = 0.15
C1 = 0.01 ** 2
C2 = 0.03 ** 2

N_CORES = 8
IMG_H = 1024
IMG_W = 1024
N_IMG_PER_CORE = 2
BLK = 126          # output rows per vertical-matmul block
MAXW_PSUM = 512    # fp32 columns per PSUM bank

MM_F32R = True     # float32r matmuls: 4x PE throughput, ~1e-6 rel error

# --- custom fused DVE ops (registered into concourse.dve_ops at build) ---- #
_OP_SQSUM = None       # out = in0^2 + in1^2
_OP_SSIM_RAT = None    # out = (in0 + s0) * (in1*s1 - in0 + imm2)
_OP_SSIM_FINAL = None  # out = (s0 - clamp(in0*in1, s1, s0))*imm2; accum += out
_CUSTOM_OPS_OK = False


def _register_custom_ops():
    global _OP_SQSUM, _OP_SSIM_RAT, _OP_SSIM_FINAL, _CUSTOM_OPS_OK
    if _CUSTOM_OPS_OK:
        return
    from operator import add

    import concourse.dve_ops as dv
    from concourse.dve_spec import (
        C0, C1 as SC1, C2 as SC2, Spec, Src0, Src1, Zero, lower, maxx, minn, sq,
    )
    from concourse.dve_uop import DveOpSpec

    def _sqsum_ref(in0, in1, c0, c1, c2):
        return in0.astype(np.float32) ** 2 + in1.astype(np.float32) ** 2

    def _rat_ref(in0, in1, c0, c1, c2):
        a = in0.astype(np.float32)
        return (a + c0) * (in1.astype(np.float32) * c1 - a + c2)

    def _final_ref(in0, in1, c0, c1, c2):
        z = in0.astype(np.float32) * in1.astype(np.float32)
        b = ((c0 - np.clip(z, c1, c0)) * c2).astype(np.float32)
        return b, b.reshape(b.shape[0], -1).sum(axis=-1, keepdims=True)

    defs = [
        ("SSIM_SQSUM_ANT", Spec(body=sq(Src0) + sq(Src1), reference=_sqsum_ref)),
        ("SSIM_RAT_ANT", Spec(
            body=(Src0 + C0) * (Src1 * SC1 - Src0 + SC2), reference=_rat_ref)),
        ("SSIM_FINAL_ANT", Spec(
            body=(C0 - maxx(minn(Src0 * Src1, C0), SC1)) * SC2,
            accum=add, accum_init=Zero, reference=_final_ref)),
    ]
    made = {}
    for name, spec in defs:
        if name not in dv._SUB_OPCODE_FOR_NAME:
            stub = dv.DveOp(name, spec, subdim=False, uops_sha={})
            dv.OPS.append(stub)
            dv._SUB_OPCODE_FOR_NAME[name] = (
                dv._CUSTOM_DVE_ROW_BASE + len(dv.OPS) - 1
            )
            dv.CUSTOM_DVE_SPECS[name] = spec
        opcode = dv._SUB_OPCODE_FOR_NAME[name]
        shas = {}
        for ver in ("v3", "v4"):
            res = DveOpSpec(
                name=name, opcode=opcode, uops=lower(spec, ver=ver),
                rd1_en=dv.has_src1(spec),
            )
            shas[ver] = res.sha(ver)
        op = dv.DveOp(name, spec, subdim=False, uops_sha=shas)
        idx = next(i for i, o in enumerate(dv.OPS) if o.name == name)
        dv.OPS[idx] = op
        dv.CUSTOM_DVE_SPECS[name] = spec
        made[name] = op
    _OP_SQSUM = made["SSIM_SQSUM_ANT"]
    _OP_SSIM_RAT = made["SSIM_RAT_ANT"]
    _OP_SSIM_FINAL = made["SSIM_FINAL_ANT"]
    _CUSTOM_OPS_OK = True


def _blocks(H):
    """Vertical block decomposition: list of (r0, n_out, rs, nr)."""
    out = []
    b = 0
    while b * BLK < H:
        r0 = b * BLK
        n_out = min(BLK, H - r0)
        rs = max(r0 - 1, 0)
        re = min(r0 + n_out, H - 1)
        out.append((r0, n_out, rs, re - rs + 1))
        b += 1
    return out


def make_bmats(H):
    """Banded vertical-sum matrices (entries 1/9), padded into [nblk,128,BLK]."""
    blocks = _blocks(H)
    bm = np.zeros((len(blocks), 128, BLK), dtype=np.float32)
    ninth = np.float32(1.0) / np.float32(9.0)
    for i, (r0, n_out, rs, nr) in enumerate(blocks):
        for k in range(nr):
            for j in range(n_out):
                if abs((rs + k) - (r0 + j)) <= 1:
                    bm[i, k, j] = ninth
    return bm


def build_program(n_img, H, W, io_internal=False):
    """Build the per-core program for n_img (even) HxW images.

    DRAM input "ptin": [ (n_img/2)*H, 4*(W+2) ] with row layout
    [0 p0 0 | 0 p1 0 | 0 t0 0 | 0 t1 0] per image pair.
    io_internal makes ptin Internal DRAM (timing-only builds).
    """
    import concourse.bacc as bacc
    import concourse.tile as tile
    from concourse import mybir

    assert n_img % 2 == 0
    f32 = mybir.dt.float32
    Alu = mybir.AluOpType
    Act = mybir.ActivationFunctionType

    blocks = _blocks(H)
    nblk = len(blocks)
    S = W + 2                       # one padded section
    S4 = 4 * S                      # packed row width
    npairs = n_img // 2
    n_chunks = (W + MAXW_PSUM - 1) // MAXW_PSUM
    W2 = 2 * W                      # field-pair width (img0|img1)

    _register_custom_ops()
    nc = bacc.Bacc("TRN2", target_bir_lowering=False, debug=False)

    io_kind = "Internal" if io_internal else "ExternalInput"
    ptin_d = nc.dram_tensor("ptin", [npairs * H, S4], f32, kind=io_kind).ap()
    bm_d = nc.dram_tensor("bmats", [nblk, 128, BLK], f32, kind="ExternalInput").ap()
    acc_d = nc.dram_tensor("acc_out", [128, 2], f32, kind="ExternalOutput").ap()

    with tile.TileContext(nc) as tc:
        with (
            tc.tile_pool(name="consts", bufs=1) as cpool,
            tc.tile_pool(name="io", bufs=2) as iopool,
            tc.tile_pool(name="hsum", bufs=2) as hpool,
            tc.tile_pool(name="post", bufs=1) as ppool,
            tc.tile_pool(name="psum", bufs=1, space="PSUM") as psumpool,
        ):
            acc = cpool.tile([128, 2], f32, tag="acc")
            nc.vector.memset(acc[:, :], 0.0)
            if io_internal:
                fill = cpool.tile([128, S4], f32, tag="fill")
                nc.vector.memset(fill[:, :], 0.625)
                rows_total = npairs * H
                for r in range(0, rows_total, 128):
                    nrr = min(128, rows_total - r)
                    nc.sync.dma_start(out=ptin_d[r:r + nrr, :], in_=fill[0:nrr, :])

            mm_dt = mybir.dt.float32r if MM_F32R else f32
            bmats = []
            for i, (r0, n_out, rs, nr) in enumerate(blocks):
                braw = cpool.tile([128, BLK], f32, tag=f"bmraw{i}", name="braw")
                nc.sync.dma_start(out=braw[0:nr, 0:n_out], in_=bm_d[i, 0:nr, 0:n_out])
                if MM_F32R:
                    bt = cpool.tile([128, BLK], mm_dt, tag=f"bmat{i}", name="bt")
                    nc.vector.tensor_copy(bt[0:nr, 0:n_out], braw[0:nr, 0:n_out])
                else:
                    bt = braw
                bmats.append(bt)

            for pair in range(npairs):
                base = pair * H
                for bi, (r0, n_out, rs, nr) in enumerate(blocks):
                    # rows [0:k_l1] of consecutive stripes tile H exactly once
                    if bi + 1 < len(blocks):
                        k_l1 = blocks[bi + 1][2] - rs
                    else:
                        k_l1 = nr

                    pt = iopool.tile([128, S4], f32, tag="pt")
                    nc.sync.dma_start(
                        out=pt[0:nr, :], in_=ptin_d[base + rs: base + rs + nr, :])

                    rows = slice(0, nr)
                    # horizontal 3-tap for p0,p1,t0,t1 in two ops
                    # (junk at section tails is never read)
                    g = hpool.tile([128, S4 - 1], f32, tag="g")
                    nc.vector.tensor_add(
                        g[rows, :], pt[rows, 0:S4 - 1], pt[rows, 1:S4])
                    h3pt = hpool.tile([128, S4 - 2], mm_dt, tag="h3pt")
                    nc.vector.tensor_add(
                        h3pt[rows, :], g[rows, 0:S4 - 2], pt[rows, 2:S4])

                    # in place: t-half <- v = p - t ; p-half <- u = 2p - v
                    nc.gpsimd.tensor_sub(
                        pt[rows, 2 * S:S4], pt[rows, 0:2 * S], pt[rows, 2 * S:S4])
                    nc.vector.scalar_tensor_tensor(
                        pt[rows, 0:2 * S], pt[rows, 0:2 * S], 2.0,
                        pt[rows, 2 * S:S4], op0=Alu.mult, op1=Alu.subtract)
                    # L1 partial: |v| in place over the disjoint-cover rows
                    l1part = ppool.tile([128, 1], f32, tag="l1part")
                    nc.scalar.activation(
                        pt[0:k_l1, 2 * S:S4], pt[0:k_l1, 2 * S:S4], Act.Abs,
                        accum_out=l1part[0:k_l1, :])
                    # squares in place: [u0 u1 v0 v1] -> [u0^2 u1^2 v0^2 v1^2]
                    nc.scalar.activation(pt[rows, :], pt[rows, :], Act.Square)

                    g2 = hpool.tile([128, S4 - 1], f32, tag="g", name="g2")
                    nc.vector.tensor_add(
                        g2[rows, :], pt[rows, 0:S4 - 1], pt[rows, 1:S4])
                    h3uv = hpool.tile([128, S4 - 2], mm_dt, tag="h3uv")
                    nc.vector.tensor_add(
                        h3uv[rows, :], g2[rows, 0:S4 - 2], pt[rows, 2:S4])

                    bmat = bmats[bi]
                    ro = slice(0, n_out)
                    pw = slice(0, W2)

                    def mm_group(h3, ps):
                        # fields [f0_img0|f0_img1|f1_img0|f1_img1] -> PSUM
                        for s in range(4):
                            for ci in range(n_chunks):
                                c0 = ci * MAXW_PSUM
                                cw = min(MAXW_PSUM, W - c0)
                                nc.tensor.matmul(
                                    ps[0:n_out, s * W + c0: s * W + c0 + cw],
                                    lhsT=bmat[0:nr, 0:n_out],
                                    rhs=h3[0:nr, s * S + c0: s * S + c0 + cw],
                                    start=True, stop=True)

                    # group 1: X|Y
                    ps = psumpool.tile([128, 4 * W], f32, tag="ps", name="ps")
                    mm_group(h3pt, ps)
                    Ysb = ppool.tile([128, W2], f32, tag="Ysb", name="Ysb")
                    nc.scalar.copy(Ysb[ro, :], ps[ro, W2:4 * W])
                    A2 = ppool.tile([128, W2], f32, tag="A2", name="A2")
                    nc.vector.scalar_tensor_tensor(
                        A2[ro, pw], ps[ro, 0:W2], 2.0, Ysb[ro, pw],
                        op0=Alu.mult, op1=Alu.mult)
                    V = ppool.tile([128, W2], f32, tag="V", name="V")
                    nc.vector._custom_dve(
                        _OP_SQSUM, out=V[ro, pw], in0=ps[ro, 0:W2], in1=Ysb[ro, pw])

                    # group 2: G|Hh (reuses the PSUM banks)
                    ps2 = psumpool.tile([128, 4 * W], f32, tag="ps", name="ps2")
                    mm_group(h3uv, ps2)
                    Hsb = ppool.tile([128, W2], f32, tag="Hsb", name="Hsb")
                    nc.scalar.copy(Hsb[ro, :], ps2[ro, W2:4 * W])
                    Dd = ppool.tile([128, W2], f32, tag="Dd", name="Dd")
                    nc.vector.tensor_sub(Dd[ro, pw], ps2[ro, 0:W2], Hsb[ro, pw])
                    M = ppool.tile([128, W2], f32, tag="M", name="M")
                    nc.vector.tensor_add(M[ro, pw], ps2[ro, 0:W2], Hsb[ro, pw])

                    # in-place: n1n2 -> A2's tile, d1d2 -> V, rcp -> M, fin -> Dd
                    n1n2 = A2
                    nc.vector._custom_dve(
                        _OP_SSIM_RAT, out=n1n2[ro, pw], in0=A2[ro, pw],
                        in1=Dd[ro, pw], s0=float(C1), s1=0.5, imm2=float(C2))
                    d1d2 = V
                    nc.vector._custom_dve(
                        _OP_SSIM_RAT, out=d1d2[ro, pw], in0=V[ro, pw],
                        in1=M[ro, pw], s0=float(C1), s1=0.5, imm2=float(C2))
                    rcp = M
                    nc.vector.reciprocal_approx_fast(rcp[ro, pw], d1d2[ro, pw])
                    fin = Dd
                    spart = ppool.tile([128, 1], f32, tag="spart")
                    nc.vector._custom_dve(
                        _OP_SSIM_FINAL, out=fin[ro, pw], in0=n1n2[ro, pw],
                        in1=rcp[ro, pw], s0=1.0, s1=-1.0, imm2=0.5,
                        accum_out=spart[ro, :])
                    nc.vector.tensor_add(
                        acc[0:n_out, 0:1], acc[0:n_out, 0:1], spart[ro, :])
                    nc.vector.tensor_add(
                        acc[0:k_l1, 1:2], acc[0:k_l1, 1:2], l1part[0:k_l1, :])

            nc.sync.dma_start(out=acc_d[:, :], in_=acc[:, :])

    nc.compile()
    return nc


_CACHE = {}


def _get_program(n_img, H, W):
    key = (n_img, H, W)
    if key not in _CACHE:
        _CACHE[key] = build_program(n_img, H, W)
    return _CACHE[key]


def _pack_inputs(pred, target):
    """pred/target [n_img, H, W] -> packed [npairs*H, 4*(W+2)]."""
    n_img, H, W = pred.shape
    assert n_img % 2 == 0
    npairs = n_img // 2
    S = W + 2
    out = np.zeros((npairs * H, 4 * S), dtype=np.float32)
    out[:, 1:W + 1] = pred[0::2].reshape(npairs * H, W)
    out[:, S + 1:S + W + 1] = pred[1::2].reshape(npairs * H, W)
    out[:, 2 * S + 1:2 * S + W + 1] = target[0::2].reshape(npairs * H, W)
    out[:, 3 * S + 1:3 * S + W + 1] = target[1::2].reshape(npairs * H, W)
    return out


LAST_RESULTS = None


def kernel(pred, target):
    from concourse.bass_utils import run_bass_kernel_spmd

    global LAST_RESULTS

    pred = np.asarray(pred, dtype=np.float32).reshape(16, IMG_H, IMG_W)
    target = np.asarray(target, dtype=np.float32).reshape(16, IMG_H, IMG_W)

    nc = _get_program(N_IMG_PER_CORE, IMG_H, IMG_W)
    bm = make_bmats(IMG_H)

    in_maps = []
    for c in range(N_CORES):
        sl = slice(c * N_IMG_PER_CORE, (c + 1) * N_IMG_PER_CORE)
        in_maps.append({
            "ptin": _pack_inputs(pred[sl], target[sl]),
            "bmats": bm,
        })

    res = run_bass_kernel_spmd(nc, in_maps, list(range(N_CORES)))
    LAST_RESULTS = res
    ssim_sum = 0.0
    l1_sum = 0.0
    for r in res.results:
        acc = r["acc_out"]
        ssim_sum += float(acc[:, 0].sum(dtype=np.float64))
        l1_sum += float(acc[:, 1].sum(dtype=np.float64))
    n = 16.0 * IMG_H * IMG_W
    loss = ALPHA * (ssim_sum / n) + BETA * (l1_sum / n)
    return np.float32(loss)
